# revision 17
# baseline (speedup 1.0000x reference)
"""Trainium2 Bass kernel for BroadcastObstaclesToLanes (embedding lookup).

out[m, :] = obs_pos[same_obs_mask[m, 0], :]   m in [0, 16777216)

Sharding: M (lanes) split across 8 NeuronCores; the obs_pos table is
replicated so every core's gather is fully local.

Per core (2,097,152 tokens), two-stage gather:
  Stage 1 (GPSIMD dma_gather, custom SWDGE ucode): the table is viewed as
  32768 blocks of 32 rows (256B). Each token fetches the 256B block
  containing its row: block id q = idx >> 5 (int16), 8192 tokens per
  instruction, token i lands at dst[i % 128, i // 128, 0:64].
  Stage 2 (DVE): within-block select o = idx & 31 via
  mask = (o == iota_pair), masked = mask * block, pair-sum over the 32
  block rows -> [128, 64, 2] f32 exact result. 3 DVE ops per chunk,
  fully hidden under the gather.
Double-buffered across 256 chunks; sync engine streams idx chunks in and
results out.
"""

import numpy as np

N_OBS = 1048576
M_LANES = 16777216
NCORES = 8
MS = M_LANES // NCORES  # 2,097,152 tokens per core
P = 128
NIDX = 8192  # tokens per dma_gather
NG = MS // NIDX  # 256 gather chunks per core
C = NIDX // P  # 64 tokens per partition per chunk
NBLK = N_OBS // 32  # 32768 blocks of 32 rows (256B each)

_cached_nc = None


def _build():
    global _cached_nc
    if _cached_nc is not None:
        return _cached_nc

    import concourse.bacc as bacc
    import concourse.bass as bass
    from concourse import mybir
    from concourse.library_config import mlp

    nc = bacc.Bacc(
        "TRN2", target_bir_lowering=False, debug=False, num_devices=NCORES
    )
    tbl = nc.dram_tensor(
        "tbl", [NBLK, 64], mybir.dt.float32, kind="ExternalInput"
    )
    q16_d = nc.dram_tensor(
        "q16", [P, MS // 16], mybir.dt.int16, kind="ExternalInput"
    )
    o_d = nc.dram_tensor(
        "off", [P, NG * C], mybir.dt.float16, kind="ExternalInput"
    )
    iota_d = nc.dram_tensor(
        "iota", [P, 64], mybir.dt.float16, kind="ExternalInput"
    )
    out = nc.dram_tensor(
        "out", [NG, P, C, 2], mybir.dt.float32, kind="ExternalOutput"
    )

    W = NIDX // 16  # idx columns per chunk
    NB = 2  # staging buffers / gathers in flight

    from contextlib import ExitStack

    with ExitStack() as _st:
        block = _st.enter_context(nc.Block())
        f32 = mybir.dt.float32

        def _sb(name, shape, dt=f32):
            return _st.enter_context(nc.sbuf_tensor(name, shape, dt))

        def _sem(name):
            return _st.enter_context(nc.semaphore(name))

        dsts = [_sb(f"dst{b}", [P, C, 64]) for b in range(NB)]
        reds = [_sb(f"red{b}", [P, C, 2]) for b in range(NB)]
        idxs = [_sb(f"idx{b}", [P, W], mybir.dt.int16) for b in range(NB)]
        msk = _sb("msk", [P, C, 64], mybir.dt.float16)
        prod = _sb("prod", [P, C, 64])
        o_sb = _sb("o_sb", [P, NG * C], mybir.dt.float16)
        iota_sb = _sb("iota_sb", [P, 64], mybir.dt.float16)
        s_pre = _sem("s_pre")
        s_idx = [_sem(f"s_idx{b}") for b in range(NB)]
        s_gat = [_sem(f"s_gat{b}") for b in range(NB)]
        s_out = [_sem(f"s_out{b}") for b in range(NB)]
        s_ext = _sem("s_ext")
        s_dve = _sem("s_dve")

        @block.sync
        def _(sy: bass.BassEngine):
            sy.dma_start(o_sb[:], o_d.ap()[:]).then_inc(s_pre, 16)
            sy.dma_start(iota_sb[:], iota_d.ap()[:]).then_inc(s_pre, 16)
            for g in range(NB):
                sy.dma_start(
                    idxs[g][:], q16_d.ap()[:, g * W : (g + 1) * W]
                ).then_inc(s_idx[g], 16)
            for g in range(NG - NB):
                sy.wait_ge(s_gat[g % NB], 16 * (g // NB + 1))
                sy.dma_start(
                    idxs[g % NB][:],
                    q16_d.ap()[:, (g + NB) * W : (g + NB + 1) * W],
                ).then_inc(s_idx[g % NB], 16)

        @block.scalar
        def _(sc: bass.BassEngine):
            for g in range(NG):
                sc.wait_ge(s_ext, g + 1)
                sc.dma_start(out.ap()[g], reds[g % NB][:]).then_inc(
                    s_out[g % NB], 16
                )

        @block.gpsimd
        def _(gp: bass.BassGpSimd):
            gp.load_library(mlp)
            for g in range(NG):
                gp.wait_ge(s_idx[g % NB], 16 * (g // NB + 1))
                if g >= NB:
                    gp.wait_ge(s_ext, g - NB + 1)
                gp.dma_gather(
                    dsts[g % NB][:], tbl.ap()[:], idxs[g % NB][:],
                    NIDX, NIDX, 64, single_packet=False,
                ).then_inc(s_gat[g % NB], 16)

        @block.vector
        def _(ve: bass.BassEngine):
            ve.wait_ge(s_pre, 32)
            for g in range(NG):
                ve.wait_ge(s_gat[g % NB], 16 * (g // NB + 1))
                if g >= NB:
                    ve.wait_ge(s_out[g % NB], 16 * (g // NB))
                o_slice = (
                    o_sb[:, g * C : (g + 1) * C]
                    .unsqueeze(2)
                    .broadcast_to([P, C, 64])
                )
                iota_b = iota_sb[:].unsqueeze(1).broadcast_to([P, C, 64])
                ve.tensor_tensor(
                    out=msk[:], in0=o_slice, in1=iota_b,
                    op=mybir.AluOpType.is_equal,
                ).then_inc(s_dve, 1)
                ve.wait_ge(s_dve, 2 * g + 1)
                ve.tensor_tensor(
                    out=prod[:], in0=msk[:], in1=dsts[g % NB][:],
                    op=mybir.AluOpType.mult,
                ).then_inc(s_dve, 1)
                ve.wait_ge(s_dve, 2 * g + 2)
                ve.tensor_reduce(
                    out=reds[g % NB][:],
                    in_=prod[:].rearrange("p c (w d) -> p c d w", w=32, d=2),
                    axis=mybir.AxisListType.X,
                    op=mybir.AluOpType.add,
                ).then_inc(s_ext, 1)

    nc.compile()
    _cached_nc = nc
    return nc


def _prepare_in_maps(obs_pos, same_obs_mask):
    tblblk = np.ascontiguousarray(
        np.asarray(obs_pos, dtype=np.float32).reshape(NBLK, 64)
    )
    idx32 = np.asarray(same_obs_mask).reshape(-1).astype(np.int32)
    iota = np.ascontiguousarray(
        np.tile((np.arange(64) // 2).astype(np.float16), (P, 1))
    )
    in_maps = []
    for c in range(NCORES):
        lanes = idx32[c * MS : (c + 1) * MS]
        q16 = (lanes >> 5).astype(np.int16)
        # wrap: token t at [t % 16, t // 16], replicated across 8 groups
        q16w = np.tile(np.ascontiguousarray(q16.reshape(MS // 16, 16).T), (8, 1))
        off = (
            (lanes & 31)
            .astype(np.float16)
            .reshape(NG, C, P)
            .transpose(2, 0, 1)
            .reshape(P, NG * C)
        )
        in_maps.append(
            {
                "tbl": tblblk,
                "q16": q16w,
                "off": np.ascontiguousarray(off),
                "iota": iota,
            }
        )
    return in_maps


def kernel(obs_pos, same_obs_mask):
    from concourse.bass_utils import run_bass_kernel_spmd

    nc = _build()
    in_maps = _prepare_in_maps(obs_pos, same_obs_mask)
    res = run_bass_kernel_spmd(nc, in_maps, core_ids=list(range(NCORES)))
    outs = []
    for r in res.results:
        o = r["out"]  # [NG, P, C, 2]; token t = g*8192 + c*128 + p
        outs.append(o.transpose(0, 2, 1, 3).reshape(MS, 2))
    return np.ascontiguousarray(np.concatenate(outs, axis=0))
